# revision 2
# baseline (speedup 1.0000x reference)
"""Trainium2 Bass kernel for the CPG actor network (nn_Actor_CPG).

Strategy (pure data parallel over 8 NeuronCores, B rows split evenly):
- Host folds every tiny CPG matrix into one fused weight W [109, 84]:
  per 128-row chunk the device runs ONE fp16 matmul out = XT_chunk.T @ W
  where XT = [obs.T; r.T; th.T; rd.T; rddo.T; ones] is host-packed
  [109, B_shard]. The 84 output columns are [m0 m1 m2 m3 | r_ddot
  r_dot r]: the theta_dot contraction pieces plus the COMPLETE
  amplitude outputs (r_ddot, r_dot, r are linear in the inputs, so the
  trapezoidal integration is folded into the weight columns).
- PSUM-reading elementwise ops run per 2048-row psum group; all
  SBUF-only fp16 elementwise ops run once per 8192-row DMA group
  (FD=768 per partition) to amortize the DVE per-op overhead.
- The [r_ddot r_dot r] triple is evacuated PSUM->SBUF by ONE grouped
  ScalarE copy per psum group; theta/theta_ddot are rebuilt from
  theta_dot and theta_dot_old in SBUF (no extra PSUM reads).
- Three independent products (r*tdo^2, rd*tdo, r*tddo) run on GpSimd
  to offload the Vector engine.
- All DRAM I/O is fp16; states also ship row-major in `nat` (the
  matmul layout is feature-major so the two layouts are disjoint
  except r/th/rd/rddo, which buy matmul columns).

Environment workarounds baked in below: the image's walrus accepts only
ONE sync-wait per instruction (Tile emits several), so the BIR is
post-processed to split waits onto single-wait Drain carriers; and the
missing antenv.axon_hooks module is shimmed.
"""
import math

import numpy as np

B, N, P, PS, OBS = 524288, 12, 24, 12, 60
DT = 0.002
NCORES = 8
BSH = B // NCORES           # 65536 rows per core
CH = 128                    # rows per matmul chunk
PGC = 16                    # chunks per PSUM group
PGROWS = CH * PGC           # 2048
NPG = BSH // PGROWS         # 32
PG_PER_DG = 4               # psum groups per DMA group
DGROWS = PGROWS * PG_PER_DG  # 8192
NDG = BSH // DGROWS         # 8
IL = (BSH // CH) * N        # 6144 interleaved free dim
DGF = IL // NDG             # 768 free per dma group
SF = DGF // PG_PER_DG       # 192 free per psum group
KX = 109                    # matmul contraction (60 obs + 4*12 state + 1)
NQ = 84                     # matmul output columns (7 quantities x 12)
NNAT = 6

# index order inside the packed nat tensor
NAT_ORDER = ["r_n", "th_n", "rd_n", "tdo_n", "rddo_n", "tddo_n"]

_cache = {}


def _split_waits_json(bir_bytes: bytes) -> bytes:
    """walrus in this image accepts ONE sync-wait per instruction; Tile
    emits several. Split them into single-wait Drains (same engine,
    program order preserved)."""
    import json
    import os
    bir = json.loads(bir_bytes)
    carrier = os.environ.get("KCARRIER", "Drain")
    for fn in bir.get("functions", []):
        for blk in fn.get("blocks", []):
            out = []
            for inst in blk.get("instructions", []):
                si = inst.get("sync_info")
                if isinstance(si, dict) and len(si.get("on_wait", [])) > 1:
                    waits = si["on_wait"]
                    for k, w in enumerate(waits[:-1]):
                        nop = {
                            "debug": inst.get("debug", 0),
                            "engine": inst["engine"],
                            "ins": [],
                            "name": f'{inst["name"]}-sw{k}',
                            "opcode": carrier,
                            "outs": [],
                            "sync_info": {"on_update": [], "on_wait": [w]},
                        }
                        if carrier == "Drain":
                            nop["is_reset_sema"] = False
                        out.append(nop)
                    si["on_wait"] = [waits[-1]]
                out.append(inst)
            blk["instructions"] = out
    return json.dumps(bir).encode()


def _install_birpatch():
    import sys
    import types
    # This image lacks antenv.axon_hooks (NTFF profiling); shim it so
    # run_bass_kernel_spmd's trace path degrades gracefully.
    if "antenv.axon_hooks" not in sys.modules:
        try:
            import antenv.axon_hooks  # noqa: F401
        except ImportError:
            mod = types.ModuleType("antenv.axon_hooks")
            mod.get_axon_ntff_profile_hook = lambda: None
            sys.modules["antenv.axon_hooks"] = mod
    from concourse import bass2jax
    if getattr(bass2jax, "_ant_birpatch_installed", False):
        return
    orig = bass2jax._decompress_ant_bir

    def patched(ant_bir_value):
        return _split_waits_json(orig(ant_bir_value))

    bass2jax._decompress_ant_bir = patched
    bass2jax._ant_birpatch_installed = True


# which SBUF-only products run on GpSimd instead of VectorE
GP_OPS = ("aa", "dd", "ee")


def _build_nc(rep=1, loop_n=None, drop=(), gp_ops=GP_OPS):
    from contextlib import nullcontext

    from concourse import bass, mybir
    from concourse.tile import TileContext

    f32, f16 = mybir.dt.float32, mybir.dt.float16
    AF = mybir.ActivationFunctionType
    OP = mybir.AluOpType

    nc = bass.Bass()

    def reg_const(value, dtype=mybir.dt.float32):
        t = nc.alloc_sbuf_tensor(f"const-{dtype.name}-{value}", [128, 1], dtype)
        nc.gpsimd.memset(t.ap(), value)
        nc.const_aps.aps[(dtype, value)] = t.ap()

    reg_const(math.pi / 2)
    nc.all_engine_barrier()

    xt_d = nc.declare_dram_parameter("xt", [KX, BSH], f16, isOutput=False)
    wm_d = nc.declare_dram_parameter("wm", [KX, NQ], f16, isOutput=False)
    nat_d = nc.declare_dram_parameter("nat", [128, NDG, NNAT, DGF], f16,
                                      isOutput=False)
    out_d = nc.declare_dram_parameter("out", [128, NDG, 9, DGF], f16,
                                      isOutput=True)

    NI = {nm: i for i, nm in enumerate(NAT_ORDER)}

    class _Null:
        def __getattr__(self, _):
            return lambda *a, **k: None

    veng = _Null() if "vec" in drop else nc.vector
    seng = _Null() if "act" in drop else nc.scalar
    geng = _Null() if "gp" in drop else nc.gpsimd
    teng = _Null() if "mm" in drop else nc.tensor

    def eng(nm):
        return geng if nm in gp_ops else veng

    with TileContext(nc) as tc:
        with tc.tile_pool(name="const", bufs=1) as cpool, \
             tc.tile_pool(name="xtp", bufs=2) as xtpool, \
             tc.tile_pool(name="natp", bufs=2) as natpool, \
             tc.tile_pool(name="outp", bufs=2) as outpool, \
             tc.tile_pool(name="midp", bufs=2) as midpool, \
             tc.tile_pool(name="psp", bufs=2, space="PSUM") as pspool:

            wm = cpool.tile([KX, NQ], f16, tag="wm")
            nc.sync.dma_start(out=wm[:, :], in_=wm_d[:, :])

            loop_cm = tc.For_i(0, loop_n, 1) if loop_n else nullcontext()
            with loop_cm:
              for dg in range(NDG * rep):
                dg = dg % NDG
                nat_t = natpool.tile([128, NNAT, DGF], f16, tag="nat",
                                     name="nat_t")
                nc.sync.dma_start(out=nat_t[:, :, :],
                                  in_=nat_d[:, dg, :, :])
                outs_t = outpool.tile([128, 9, DGF], f16, tag="outs",
                                      name="outs_t")
                xt = xtpool.tile([KX, DGROWS], f16, tag="xt", name="xt")
                nc.sync.dma_start(
                    out=xt[:, :],
                    in_=xt_d[:, dg * DGROWS:(dg + 1) * DGROWS])

                def nv(nm):  # [128, 768] per-dg state view
                    return nat_t[:, NI[nm], :]

                def ov(q):  # [128, 768] per-dg output plane view
                    return outs_t[:, q, :]

                def mid(nm):
                    t = midpool.tile([128, DGF], f16, tag=nm, name=nm)
                    return t[:, :]

                # per-dg ScalarE transcendentals from nat (FD=768)
                cos_t, sin_t, tdo2 = mid("cos_t"), mid("sin_t"), mid("tdo2")
                seng.activation(cos_t, nv("th_n"), AF.Sin, bias=math.pi / 2)
                seng.activation(sin_t, nv("th_n"), AF.Sin)
                seng.activation(tdo2, nv("tdo_n"), AF.Square)

                slm, p1m, p2m, t6m = (mid("slm"), mid("p1m"), mid("p2m"),
                                      mid("t6m"))

                for s in range(PG_PER_DG):
                    ps = pspool.tile([128, PGC, 128], f32, tag="ps", name="ps")
                    for c in range(PGC):
                        teng.matmul(
                            out=ps[:, c, 0:NQ],
                            lhsT=xt[:, (s * PGC + c) * CH:
                                    (s * PGC + c + 1) * CH],
                            rhs=wm[:, :],
                            start=True, stop=True)

                    def m(q):  # [128, 16, 12] psum quantity view
                        return ps[:, :, q * N:(q + 1) * N]

                    def sv3(ap):  # [128, 768] mid -> [128, 16, 12] s-slice
                        return ap[:, s * SF:(s + 1) * SF].rearrange(
                            "p (a b) -> p a b", a=PGC)

                    # ScalarE: sin(m3) + grouped [r_ddot r_dot r] copy-out
                    seng.activation(sv3(slm), m(3), AF.Sin)
                    seng.activation(
                        outs_t[:, 6:9, s * SF:(s + 1) * SF].rearrange(
                            "p q (c n) -> p c q n", c=PGC),
                        ps[:, :, 4 * N:7 * N].rearrange(
                            "p c (q n) -> p c q n", q=3),
                        AF.Copy)
                    # VectorE psum-side: p1 = m2*sl, p2 = m1*cos, t6 = m0+p1
                    veng.tensor_tensor(sv3(p1m), m(2), sv3(slm), OP.mult)
                    veng.tensor_tensor(sv3(p2m), m(1), sv3(cos_t), OP.mult)
                    veng.tensor_tensor(sv3(t6m), m(0), sv3(p1m), OP.add)

                # per-dg SBUF-only elementwise (FD=768)
                # theta_dot = t6 - p2
                veng.tensor_tensor(ov(4), t6m, p2m, OP.subtract)
                # theta = th + (td + tdo)*DT/2 ; thdd = (td - tdo)/DT
                thv, thv2 = mid("thv"), mid("thv2")
                veng.tensor_tensor(thv, ov(4), nv("tdo_n"), OP.subtract)
                veng.tensor_scalar_mul(ov(5), thv, 1.0 / DT)
                veng.tensor_tensor(thv2, ov(4), nv("tdo_n"), OP.add)
                veng.scalar_tensor_tensor(
                    ov(3), thv2, DT / 2, nv("th_n"), OP.mult, OP.add)
                # x = r*cos ; x_dot = rd*cos - r*sin*tdo
                st, rc, qq = mid("st"), mid("rc"), mid("qq")
                veng.tensor_tensor(ov(0), nv("r_n"), cos_t, OP.mult)
                veng.tensor_tensor(st, sin_t, nv("tdo_n"), OP.mult)
                veng.tensor_tensor(rc, nv("rd_n"), cos_t, OP.mult)
                veng.tensor_tensor(qq, nv("r_n"), st, OP.mult)
                veng.tensor_tensor(ov(1), rc, qq, OP.subtract)
                # x_dd = cos*(rddo - r*tdo^2) - sin*(2*rd*tdo + r*tddo)
                aa, bb, cc = mid("aa"), mid("bb"), mid("cc")
                dd, ee, ff, gg = (mid("dd"), mid("ee"),
                                  mid("ff"), mid("gg"))
                eng("aa").tensor_tensor(aa, nv("r_n"), tdo2, OP.mult)
                eng("bb").tensor_tensor(bb, nv("rddo_n"), aa, OP.subtract)
                eng("cc").tensor_tensor(cc, cos_t, bb, OP.mult)
                eng("dd").tensor_tensor(dd, nv("rd_n"), nv("tdo_n"), OP.mult)
                eng("ee").tensor_tensor(ee, nv("r_n"), nv("tddo_n"), OP.mult)
                eng("ff").scalar_tensor_tensor(ff, dd, 2.0, ee,
                                               OP.mult, OP.add)
                eng("gg").tensor_tensor(gg, sin_t, ff, OP.mult)
                veng.tensor_tensor(ov(2), cc, gg, OP.subtract)

                if "store" not in drop:
                    nc.sync.dma_start(out=out_d[:, dg, :, :],
                                      in_=outs_t[:, :, :])
    return nc


def _fold_weights(inp):
    """Host-side constant folding -> W [109, 84] fp16 (fp64 math).

    Columns: m0 (2pi(Cdv*Dd+Odv)), m1 (sigma term), m2 (Wv*lam_r),
    m3 (lam_th - Fiv), m4 = r_ddot, m5 = r_dot, m6 = r  (all complete).
    """
    g = {k: np.asarray(inp[k], np.float64) for k in
         ("v_short", "sym", "fixed", "Wd", "Ws", "Cd", "Od", "W", "Fi", "A",
          "Cr", "Or", "Lambda", "Lambda_T", "SIGMA", "D")}
    v = g["sym"] @ g["v_short"] + g["fixed"]
    Cdv, Odv = g["Cd"] @ v, g["Od"] @ v
    Wv, Fiv = g["W"] @ v, g["Fi"] @ v
    Av, Crv, Orv = g["A"] @ v, g["Cr"] @ v, g["Or"] @ v
    DWd = g["D"] @ g["Wd"]          # [12, 60]
    SWs = g["SIGMA"] @ g["Ws"]      # [12, 60]
    Lmd = g["Lambda"] - g["Lambda_T"]
    AvSq4 = Av * Av / 4.0
    a1, a0v = AvSq4 * Crv, AvSq4 * Orv

    W = np.zeros((KX, NQ), np.float64)
    two_pi = 2.0 * math.pi
    r0, rr, rth, rrd, rrddo, rone = 0, 60, 72, 84, 96, 108
    for n in range(N):
        W[r0:r0 + 60, n] = two_pi * Cdv[n] * DWd[n]
        W[rone, n] = two_pi * Odv[n]
        W[r0:r0 + 60, 12 + n] = SWs[n]
        W[rr:rr + 12, 24 + n] = Wv[n] * g["Lambda"][n]
        W[rth:rth + 12, 36 + n] = Lmd[n]
        W[rone, 36 + n] = -Fiv[n]
        # m4 = r_ddot
        W[r0:r0 + 60, 48 + n] = a1[n] * DWd[n]
        W[rone, 48 + n] = a0v[n]
        W[rr + n, 48 + n] = -AvSq4[n]
        W[rrd + n, 48 + n] = -Av[n]
        # m5 = r_dot = rd + (rddo + r_ddot)*DT/2
        W[:, 60 + n] = (DT / 2) * W[:, 48 + n]
        W[rrd + n, 60 + n] += 1.0
        W[rrddo + n, 60 + n] += DT / 2
        # m6 = r = r_old + rd*DT + (rddo + r_ddot)*DT^2/4
        W[:, 72 + n] = (DT * DT / 4) * W[:, 48 + n]
        W[rr + n, 72 + n] += 1.0
        W[rrd + n, 72 + n] += DT
        W[rrddo + n, 72 + n] += DT * DT / 4
    return W.astype(np.float16)


def _interleave(arr):
    """[BSH, N] -> [128, IL] so each partition holds its own rows."""
    return arr.reshape(BSH // CH, CH, N).transpose(1, 0, 2).reshape(128, IL)


def _prepare_in_maps(inputs):
    inp = {k: np.asarray(v) for k, v in inputs.items()}
    Wm = _fold_weights(inp)

    obs = np.asarray(inp["obs"], np.float32)
    states = {k: np.asarray(inp[k], np.float32) for k in
              ("theta_old", "theta_dot_old", "theta_dot_dot_old",
               "r_old", "r_dot_old", "r_dot_dot_old")}
    nat_src = {"r_n": "r_old", "th_n": "theta_old", "rd_n": "r_dot_old",
               "tdo_n": "theta_dot_old", "rddo_n": "r_dot_dot_old",
               "tddo_n": "theta_dot_dot_old"}

    in_maps = []
    for i in range(NCORES):
        sl = slice(i * BSH, (i + 1) * BSH)
        xt = np.empty((KX, BSH), np.float16)
        xt[0:60] = obs[sl].T
        xt[60:72] = states["r_old"][sl].T
        xt[72:84] = states["theta_old"][sl].T
        xt[84:96] = states["r_dot_old"][sl].T
        xt[96:108] = states["r_dot_dot_old"][sl].T
        xt[108] = 1.0
        # nat: [128, NDG, NNAT, DGF] fp16
        il = np.stack([_interleave(states[nat_src[nm]][sl])
                       for nm in NAT_ORDER])          # [6, 128, IL]
        nat = np.ascontiguousarray(
            il.reshape(NNAT, 128, NDG, DGF).transpose(1, 2, 0, 3)
        ).astype(np.float16)
        in_maps.append({"xt": xt, "wm": Wm, "nat": nat})
    return in_maps


# device plane order -> reference plane order
# device: [x, x_dot, x_ddot, theta, theta_dot, theta_ddot, r_ddot, r_dot, r]
PLANE_PERM = [0, 1, 2, 3, 4, 5, 8, 7, 6]


def kernel(**inputs):
    _install_birpatch()
    from concourse.bass_utils import run_bass_kernel_spmd

    in_maps = _prepare_in_maps(inputs)

    if "nc" not in _cache:
        _cache["nc"] = _build_nc()
    nc = _cache["nc"]

    res = run_bass_kernel_spmd(nc, in_maps, core_ids=list(range(NCORES)))

    out = np.empty((9, B, N), np.float32)
    for i in range(NCORES):
        o = res.results[i]["out"].astype(np.float32)  # [128, NDG, 9, DGF]
        # -> [9, 128, IL]: invert the per-dg packing
        o = o.transpose(2, 0, 1, 3).reshape(9, 128, IL)
        o = o.reshape(9, 128, BSH // CH, N).transpose(0, 2, 1, 3)
        out[:, i * BSH:(i + 1) * BSH] = o.reshape(9, BSH, N)[PLANE_PERM]
    return out


# revision 12
# speedup vs baseline: 4.2153x; 4.2153x over previous
"""Trainium2 Bass kernel for the CPG actor network (nn_Actor_CPG).

Strategy (pure data parallel over 8 NeuronCores, B rows split evenly):
- Host folds every tiny CPG matrix into one fused weight W [109, 84]:
  per 128-row chunk the device runs ONE fp16 matmul out = XT_chunk.T @ W
  where XT = [obs.T; r.T; th.T; rd.T; rddo.T; ones] is host-packed
  [109, B_shard]. The 84 output columns are [m0 m1 m2 m3 | r_ddot
  r_dot r]: the theta_dot contraction pieces plus the COMPLETE
  amplitude outputs (r_ddot, r_dot, r are linear in the inputs, so the
  trapezoidal integration is folded into the weight columns).
- PSUM-reading elementwise ops run per 2048-row psum group; all
  SBUF-only fp16 elementwise ops run once per 8192-row DMA group
  (FD=768 per partition) to amortize the DVE per-op overhead.
- The [r_ddot r_dot r] triple is evacuated PSUM->SBUF by ONE grouped
  ScalarE copy per psum group; theta/theta_ddot are rebuilt from
  theta_dot and theta_dot_old in SBUF (no extra PSUM reads).
- Three independent products (r*tdo^2, rd*tdo, r*tddo) run on GpSimd
  to offload the Vector engine.
- All DRAM I/O is fp16; states also ship row-major in `nat` (the
  matmul layout is feature-major so the two layouts are disjoint
  except r/th/rd/rddo, which buy matmul columns).

Environment workarounds baked in below: the image's walrus accepts only
ONE sync-wait per instruction (Tile emits several), so the BIR is
post-processed to split waits onto single-wait Drain carriers; and the
missing antenv.axon_hooks module is shimmed.
"""
import math

import numpy as np

B, N, P, PS, OBS = 524288, 12, 24, 12, 60
DT = 0.002
NCORES = 8
BSH = B // NCORES           # 65536 rows per core
CH = 128                    # rows per matmul chunk
PGC = 16                    # chunks per PSUM group
PGROWS = CH * PGC           # 2048
NPG = BSH // PGROWS         # 32
PG_PER_DG = 4               # psum groups per DMA group
DGROWS = PGROWS * PG_PER_DG  # 8192
NDG = BSH // DGROWS         # 8
IL = (BSH // CH) * N        # 6144 interleaved free dim
DGF = IL // NDG             # 768 free per dma group
SF = DGF // PG_PER_DG       # 192 free per psum group
KX = 109                    # matmul contraction (60 obs + 4*12 state + 1)
KXP = 128                   # xt partition pad: only 128-partition DMA tiles
                            # spread across all 16 SDMA engines (109 -> 27GB/s)
NQ = 84                     # matmul output columns (7 quantities x 12)
NNAT = 6

# index order inside the packed nat tensor
NAT_ORDER = ["r_n", "th_n", "rd_n", "tdo_n", "rddo_n", "tddo_n"]

_cache = {}


def _split_waits_json(bir_bytes: bytes) -> bytes:
    """walrus in this image accepts ONE sync-wait per instruction; Tile
    emits several. Split them into single-wait Drains (same engine,
    program order preserved)."""
    import json
    import os
    bir = json.loads(bir_bytes)
    carrier = os.environ.get("KCARRIER", "Drain")
    for fn in bir.get("functions", []):
        for blk in fn.get("blocks", []):
            out = []
            for inst in blk.get("instructions", []):
                si = inst.get("sync_info")
                if isinstance(si, dict) and len(si.get("on_wait", [])) > 1:
                    waits = si["on_wait"]
                    for k, w in enumerate(waits[:-1]):
                        nop = {
                            "debug": inst.get("debug", 0),
                            "engine": inst["engine"],
                            "ins": [],
                            "name": f'{inst["name"]}-sw{k}',
                            "opcode": carrier,
                            "outs": [],
                            "sync_info": {"on_update": [], "on_wait": [w]},
                        }
                        if carrier == "Drain":
                            nop["is_reset_sema"] = False
                        out.append(nop)
                    si["on_wait"] = [waits[-1]]
                out.append(inst)
            blk["instructions"] = out
    return json.dumps(bir).encode()


def _install_birpatch():
    import sys
    import types
    # This image lacks antenv.axon_hooks (NTFF profiling); shim it so
    # run_bass_kernel_spmd's trace path degrades gracefully.
    if "antenv.axon_hooks" not in sys.modules:
        try:
            import antenv.axon_hooks  # noqa: F401
        except ImportError:
            mod = types.ModuleType("antenv.axon_hooks")
            mod.get_axon_ntff_profile_hook = lambda: None
            sys.modules["antenv.axon_hooks"] = mod
    from concourse import bass2jax
    if getattr(bass2jax, "_ant_birpatch_installed", False):
        return
    orig = bass2jax._decompress_ant_bir

    def patched(ant_bir_value):
        return _split_waits_json(orig(ant_bir_value))

    bass2jax._decompress_ant_bir = patched
    bass2jax._ant_birpatch_installed = True


# which SBUF-only products run on GpSimd instead of VectorE
# (measured: GpSimd offload loses to keeping everything on VectorE)
GP_OPS = ()


def _build_nc(rep=1, loop_n=None, drop=(), gp_ops=GP_OPS, kxp=KXP,
              store_eng='act', evac3=True,
              midb=3, xtb=2, natb=2, outb=3):
    from contextlib import nullcontext

    from concourse import bass, mybir
    from concourse.tile import TileContext

    f32, f16 = mybir.dt.float32, mybir.dt.float16
    AF = mybir.ActivationFunctionType
    OP = mybir.AluOpType

    nc = bass.Bass()

    def reg_const(value, dtype=mybir.dt.float32):
        t = nc.alloc_sbuf_tensor(f"const-{dtype.name}-{value}", [128, 1], dtype)
        nc.gpsimd.memset(t.ap(), value)
        nc.const_aps.aps[(dtype, value)] = t.ap()

    reg_const(math.pi / 2)
    nc.all_engine_barrier()

    xt_d = nc.declare_dram_parameter("xt", [kxp, BSH], f16, isOutput=False)
    wm_d = nc.declare_dram_parameter("wm", [kxp, NQ], f16, isOutput=False)
    nat_d = nc.declare_dram_parameter("nat", [128, NDG, NNAT, DGF], f16,
                                      isOutput=False)
    out_d = nc.declare_dram_parameter("out", [128, NDG, 9, DGF], f16,
                                      isOutput=True)

    NI = {nm: i for i, nm in enumerate(NAT_ORDER)}

    class _Null:
        def __getattr__(self, _):
            return lambda *a, **k: None

    veng = _Null() if "vec" in drop else nc.vector
    seng = _Null() if "act" in drop else nc.scalar
    geng = _Null() if "gp" in drop else nc.gpsimd
    teng = _Null() if "mm" in drop else nc.tensor

    def eng(nm):
        return geng if nm in gp_ops else veng

    with TileContext(nc) as tc:
        with tc.tile_pool(name="const", bufs=1) as cpool, \
             tc.tile_pool(name="xtp", bufs=xtb) as xtpool, \
             tc.tile_pool(name="natp", bufs=natb) as natpool, \
             tc.tile_pool(name="outp", bufs=outb) as outpool, \
             tc.tile_pool(name="midp", bufs=midb) as midpool, \
             tc.tile_pool(name="psp", bufs=2, space="PSUM") as pspool:

            wm = cpool.tile([kxp, NQ], f16, tag="wm")
            nc.sync.dma_start(out=wm[:, :], in_=wm_d[:, :])

            loop_cm = tc.For_i(0, loop_n, 1) if loop_n else nullcontext()
            with loop_cm:
              for dg in range(NDG * rep):
                dg = dg % NDG
                nat_t = natpool.tile([128, NNAT, DGF], f16, tag="nat",
                                     name="nat_t")
                if "natload" not in drop:
                    nc.sync.dma_start(out=nat_t[:, :, :],
                                      in_=nat_d[:, dg, :, :])
                outs_t = outpool.tile([128, 9, DGF], f16, tag="outs",
                                      name="outs_t")
                xt = xtpool.tile([kxp, DGROWS], f16, tag="xt", name="xt")
                if "xtload" not in drop:
                    nc.sync.dma_start(
                        out=xt[:, :],
                        in_=xt_d[:, dg * DGROWS:(dg + 1) * DGROWS])

                def nv(nm):  # [128, 768] per-dg state view
                    return nat_t[:, NI[nm], :]

                def ov(q):  # [128, 768] per-dg output plane view
                    return outs_t[:, q, :]

                def mid(nm):
                    t = midpool.tile([128, DGF], f16, tag=nm, name=nm)
                    return t[:, :]

                # per-dg ScalarE transcendentals from nat (FD=768)
                cos_t, sin_t, tdo2 = mid("cos_t"), mid("sin_t"), mid("tdo2")
                seng.activation(cos_t, nv("th_n"), AF.Sin, bias=math.pi / 2)
                seng.activation(sin_t, nv("th_n"), AF.Sin)
                seng.activation(tdo2, nv("tdo_n"), AF.Square)

                slm, p1m, p2m, t6m = (mid("slm"), mid("p1m"), mid("p2m"),
                                      mid("t6m"))
                if evac3:
                    m012 = midpool.tile([128, PG_PER_DG, PGC, 3 * N], f16,
                                        tag="m012", name="m012")

                for s in range(PG_PER_DG):
                    ps = pspool.tile([128, PGC, 128], f32, tag="ps", name="ps")
                    for c in range(PGC):
                        teng.matmul(
                            out=ps[:, c, 0:NQ],
                            lhsT=xt[:, (s * PGC + c) * CH:
                                    (s * PGC + c + 1) * CH],
                            rhs=wm[:, :],
                            start=True, stop=True)

                    def m(q):  # [128, 16, 12] psum quantity view
                        return ps[:, :, q * N:(q + 1) * N]

                    def sv3(ap):  # [128, 768] mid -> [128, 16, 12] s-slice
                        return ap[:, s * SF:(s + 1) * SF].rearrange(
                            "p (a b) -> p a b", a=PGC)

                    # ScalarE: sin(m3) + grouped [r_ddot r_dot r] copy-out
                    seng.activation(sv3(slm), m(3), AF.Sin)
                    seng.activation(
                        outs_t[:, 6:9, s * SF:(s + 1) * SF].rearrange(
                            "p q (c n) -> p c q n", c=PGC),
                        ps[:, :, 4 * N:7 * N].rearrange(
                            "p c (q n) -> p c q n", q=3),
                        AF.Copy)
                    if evac3:
                        # grouped [m0 m1 m2] evacuation; psum ops move off DVE
                        seng.activation(m012[:, s, :, :],
                                        ps[:, :, 0:3 * N], AF.Copy)
                    else:
                        # VectorE psum-side: p1=m2*sl, p2=m1*cos, t6=m0+p1
                        veng.tensor_tensor(sv3(p1m), m(2), sv3(slm), OP.mult)
                        veng.tensor_tensor(sv3(p2m), m(1), sv3(cos_t), OP.mult)
                        veng.tensor_tensor(sv3(t6m), m(0), sv3(p1m), OP.add)

                # per-dg SBUF-only elementwise (FD=768)
                if evac3:
                    def mf(q):  # [128, 4, 16, 12] fp16 quantity view
                        return m012[:, :, :, q * N:(q + 1) * N]

                    def r4(ap):  # [128, 768] -> [128, 4, 16, 12]
                        return ap.rearrange("p (s a b) -> p s a b",
                                            s=PG_PER_DG, a=PGC)
                    veng.tensor_tensor(r4(p1m), mf(2), r4(slm), OP.mult)
                    veng.tensor_tensor(r4(p2m), mf(1), r4(cos_t), OP.mult)
                    veng.tensor_tensor(r4(t6m), mf(0), r4(p1m), OP.add)
                # theta_dot = t6 - p2
                veng.tensor_tensor(ov(4), t6m, p2m, OP.subtract)
                # theta = th + (td + tdo)*DT/2 ; thdd = (td - tdo)/DT
                thv, thv2 = mid("thv"), mid("thv2")
                veng.tensor_tensor(thv, ov(4), nv("tdo_n"), OP.subtract)
                veng.tensor_scalar_mul(ov(5), thv, 1.0 / DT)
                veng.tensor_tensor(thv2, ov(4), nv("tdo_n"), OP.add)
                veng.scalar_tensor_tensor(
                    ov(3), thv2, DT / 2, nv("th_n"), OP.mult, OP.add)
                # x = r*cos ; x_dot = rd*cos - r*sin*tdo
                st, rc, qq = mid("st"), mid("rc"), mid("qq")
                veng.tensor_tensor(ov(0), nv("r_n"), cos_t, OP.mult)
                veng.tensor_tensor(st, sin_t, nv("tdo_n"), OP.mult)
                veng.tensor_tensor(rc, nv("rd_n"), cos_t, OP.mult)
                veng.tensor_tensor(qq, nv("r_n"), st, OP.mult)
                veng.tensor_tensor(ov(1), rc, qq, OP.subtract)
                # x_dd = cos*(rddo - r*tdo^2) - sin*(2*rd*tdo + r*tddo)
                aa, bb, cc = mid("aa"), mid("bb"), mid("cc")
                dd, ee, ff, gg = (mid("dd"), mid("ee"),
                                  mid("ff"), mid("gg"))
                eng("aa").tensor_tensor(aa, nv("r_n"), tdo2, OP.mult)
                eng("bb").tensor_tensor(bb, nv("rddo_n"), aa, OP.subtract)
                eng("cc").tensor_tensor(cc, cos_t, bb, OP.mult)
                eng("dd").tensor_tensor(dd, nv("rd_n"), nv("tdo_n"), OP.mult)
                eng("ee").tensor_tensor(ee, nv("r_n"), nv("tddo_n"), OP.mult)
                eng("ff").scalar_tensor_tensor(ff, dd, 2.0, ee,
                                               OP.mult, OP.add)
                eng("gg").tensor_tensor(gg, sin_t, ff, OP.mult)
                veng.tensor_tensor(ov(2), cc, gg, OP.subtract)

                if "store" not in drop:
                    # ACT-issued HWDGE ring: stores must not head-of-line
                    # block the next dg's loads on the SP ring
                    deng = nc.scalar if store_eng == "act" else nc.sync
                    deng.dma_start(out=out_d[:, dg, :, :],
                                   in_=outs_t[:, :, :])
    return nc


def _fold_weights(inp):
    """Host-side constant folding -> W [109, 84] fp16 (fp64 math).

    Columns: m0 (2pi(Cdv*Dd+Odv)), m1 (sigma term), m2 (Wv*lam_r),
    m3 (lam_th - Fiv), m4 = r_ddot, m5 = r_dot, m6 = r  (all complete).
    """
    g = {k: np.asarray(inp[k], np.float64) for k in
         ("v_short", "sym", "fixed", "Wd", "Ws", "Cd", "Od", "W", "Fi", "A",
          "Cr", "Or", "Lambda", "Lambda_T", "SIGMA", "D")}
    v = g["sym"] @ g["v_short"] + g["fixed"]
    Cdv, Odv = g["Cd"] @ v, g["Od"] @ v
    Wv, Fiv = g["W"] @ v, g["Fi"] @ v
    Av, Crv, Orv = g["A"] @ v, g["Cr"] @ v, g["Or"] @ v
    DWd = g["D"] @ g["Wd"]          # [12, 60]
    SWs = g["SIGMA"] @ g["Ws"]      # [12, 60]
    Lmd = g["Lambda"] - g["Lambda_T"]
    AvSq4 = Av * Av / 4.0
    a1, a0v = AvSq4 * Crv, AvSq4 * Orv

    W = np.zeros((KXP, NQ), np.float64)
    two_pi = 2.0 * math.pi
    r0, rr, rth, rrd, rrddo, rone = 0, 60, 72, 84, 96, 108
    for n in range(N):
        W[r0:r0 + 60, n] = two_pi * Cdv[n] * DWd[n]
        W[rone, n] = two_pi * Odv[n]
        W[r0:r0 + 60, 12 + n] = SWs[n]
        W[rr:rr + 12, 24 + n] = Wv[n] * g["Lambda"][n]
        W[rth:rth + 12, 36 + n] = Lmd[n]
        W[rone, 36 + n] = -Fiv[n]
        # m4 = r_ddot
        W[r0:r0 + 60, 48 + n] = a1[n] * DWd[n]
        W[rone, 48 + n] = a0v[n]
        W[rr + n, 48 + n] = -AvSq4[n]
        W[rrd + n, 48 + n] = -Av[n]
        # m5 = r_dot = rd + (rddo + r_ddot)*DT/2
        W[:, 60 + n] = (DT / 2) * W[:, 48 + n]
        W[rrd + n, 60 + n] += 1.0
        W[rrddo + n, 60 + n] += DT / 2
        # m6 = r = r_old + rd*DT + (rddo + r_ddot)*DT^2/4
        W[:, 72 + n] = (DT * DT / 4) * W[:, 48 + n]
        W[rr + n, 72 + n] += 1.0
        W[rrd + n, 72 + n] += DT
        W[rrddo + n, 72 + n] += DT * DT / 4
    return W.astype(np.float16)


def _interleave(arr):
    """[BSH, N] -> [128, IL] so each partition holds its own rows."""
    return arr.reshape(BSH // CH, CH, N).transpose(1, 0, 2).reshape(128, IL)


def _prepare_in_maps(inputs):
    inp = {k: np.asarray(v) for k, v in inputs.items()}
    Wm = _fold_weights(inp)

    obs = np.asarray(inp["obs"], np.float32)
    states = {k: np.asarray(inp[k], np.float32) for k in
              ("theta_old", "theta_dot_old", "theta_dot_dot_old",
               "r_old", "r_dot_old", "r_dot_dot_old")}
    nat_src = {"r_n": "r_old", "th_n": "theta_old", "rd_n": "r_dot_old",
               "tdo_n": "theta_dot_old", "rddo_n": "r_dot_dot_old",
               "tddo_n": "theta_dot_dot_old"}

    in_maps = []
    for i in range(NCORES):
        sl = slice(i * BSH, (i + 1) * BSH)
        xt = np.zeros((KXP, BSH), np.float16)
        xt[0:60] = obs[sl].T
        xt[60:72] = states["r_old"][sl].T
        xt[72:84] = states["theta_old"][sl].T
        xt[84:96] = states["r_dot_old"][sl].T
        xt[96:108] = states["r_dot_dot_old"][sl].T
        xt[108] = 1.0
        # nat: [128, NDG, NNAT, DGF] fp16
        il = np.stack([_interleave(states[nat_src[nm]][sl])
                       for nm in NAT_ORDER])          # [6, 128, IL]
        nat = np.ascontiguousarray(
            il.reshape(NNAT, 128, NDG, DGF).transpose(1, 2, 0, 3)
        ).astype(np.float16)
        in_maps.append({"xt": xt, "wm": Wm, "nat": nat})
    return in_maps


# device plane order -> reference plane order
# device: [x, x_dot, x_ddot, theta, theta_dot, theta_ddot, r_ddot, r_dot, r]
PLANE_PERM = [0, 1, 2, 3, 4, 5, 8, 7, 6]


def kernel(**inputs):
    _install_birpatch()
    from concourse.bass_utils import run_bass_kernel_spmd

    in_maps = _prepare_in_maps(inputs)

    if "nc" not in _cache:
        _cache["nc"] = _build_nc()
    nc = _cache["nc"]

    res = run_bass_kernel_spmd(nc, in_maps, core_ids=list(range(NCORES)))

    out = np.empty((9, B, N), np.float32)
    for i in range(NCORES):
        o = res.results[i]["out"].astype(np.float32)  # [128, NDG, 9, DGF]
        # -> [9, 128, IL]: invert the per-dg packing
        o = o.transpose(2, 0, 1, 3).reshape(9, 128, IL)
        o = o.reshape(9, 128, BSH // CH, N).transpose(0, 2, 1, 3)
        out[:, i * BSH:(i + 1) * BSH] = o.reshape(9, BSH, N)[PLANE_PERM]
    return out
